# revision 13
# baseline (speedup 1.0000x reference)
"""Dense linear layer out = x @ W.T + b on 8 Trainium2 NeuronCores.

Strategy: data-parallel over the batch dim (8192/8 = 1024 rows per core),
W replicated, plus single-level Strassen per core to cut TensorE work to
7/8 of the naive roofline (the baseline bf16 kernel already ran at ~95%
of the 437us/core peak, so fewer MACs is the only lever left).

Per core, with 2x2x2 blocking of [1024,4096]@[4096,4096]:
    M1=(X11+X22)(W11+W22)  M2=(X21+X22)W11  M3=X11(W12-W22)
    M4=X22(W21-W11)        M5=(X11+X12)W22  M6=(X21-X11)(W11+W12)
    M7=(X12-X22)(W21+W22)
    C11=M1+M4-M5+M7  C12=M3+M5  C21=M2+M4  C22=M1-M2+M3+M6

All 7 lhs operands L_i (x-side sums, [2048,512] each) and 7 rhs operands
R_i (W-side sums, [2048,2048]) are precomputed on the HOST in fp32 and
shipped as bf16 - the device never does input additions. L is SBUF
resident (112KB/partition-col); R streams through a 3-slot slab ring.
Each M_i[t] tile is accumulated in PSUM (16-matmul chain) and folded
into the C-block SBUF accumulators by vector-engine ops directly from
PSUM (bias is folded into the first contribution), so M_i is never
stored. 1792 matmuls of [128x128]@[128x512] vs 2048 for the direct
kernel. Error ~4.6e-3 (bf16 operand sums double the bf16 noise).
"""

import numpy as np
import ml_dtypes

B, IN, OUT = 8192, 4096, 4096
NCORES = 8
MS = B // NCORES    # 1024 batch rows per core

P = 128
NF = 512            # matmul moving free dim (one PSUM bank of fp32)
M2 = MS // 2        # 512   Strassen half-M
K2 = IN // 2        # 2048  half-K
N2 = OUT // 2       # 2048  half-N
KT = K2 // P        # 16 contraction k-tiles per product
MT2 = M2 // P       # 4 m-tiles per half
NSL = N2 // NF      # 4 column slabs per product

_cache = {}

# per-product contributions: i -> list of (block, action)
# actions: 'init' = C := psum + bias ; 'add'/'sub' = C op= psum;
# a trailing '!' marks the block complete -> DMA out
_CONTRIB = {
    0: [("C11", "init"), ("C22", "init")],
    1: [("C21", "init"), ("C22", "sub")],
    2: [("C12", "init"), ("C22", "add")],
    3: [("C11", "add"), ("C21", "add!")],
    4: [("C11", "sub"), ("C12", "add!")],
    5: [("C22", "add!")],
    6: [("C11", "add!")],
}
# block -> (m-half, n-half)
_BLOCK_POS = {"C11": (0, 0), "C12": (0, 1), "C21": (1, 0), "C22": (1, 1)}


def _build():
    import concourse.mybir as mybir
    import concourse.tile as tile
    from concourse import bacc

    bf16 = mybir.dt.bfloat16
    f32 = mybir.dt.float32
    ADD = mybir.AluOpType.add
    SUB = mybir.AluOpType.subtract

    nc = bacc.Bacc("TRN2", target_bir_lowering=False, debug=False,
                   num_devices=NCORES)
    lt = nc.dram_tensor("lt", [7, K2, M2], bf16, kind="ExternalInput")
    rt = nc.dram_tensor("rt", [7, K2, N2], bf16, kind="ExternalInput")
    bb = nc.dram_tensor("bb", [P, OUT], bf16, kind="ExternalInput")
    out = nc.dram_tensor("out", [MS, OUT], f32, kind="ExternalOutput")

    lt_t = lt[:].rearrange("i (kt p) m -> p i kt m", p=P)  # [128,7,KT,M2]
    rt_t = rt[:].rearrange("i (kt p) n -> p i kt n", p=P)  # [128,7,KT,N2]
    out_t = out[:].rearrange("(mt p) n -> p mt n", p=P)    # [128,8,OUT]

    steps = [(ns, i) for ns in range(NSL) for i in range(7)]

    with tile.TileContext(nc) as tc:
        with (
            tc.tile_pool(name="lres", bufs=1) as lres_pool,
            tc.tile_pool(name="bias", bufs=1) as bias_pool,
            tc.tile_pool(name="wts", bufs=3) as wts_pool,
            tc.tile_pool(name="psum", bufs=8, space="PSUM") as psum_pool,
            tc.tile_pool(name="cacc", bufs=16) as cacc_pool,
        ):
            lres = lres_pool.tile([P, 7, KT, M2], bf16)   # 112KB/partition
            bias = bias_pool.tile([P, OUT], bf16)

            # PE warmup: burn the NEFF-preamble + first-DMA window with
            # dummy matmuls so the clock gate opens before the real stream.
            wz = bias_pool.tile([P, NF], bf16, name="wz")
            nc.vector.memset(wz[:], 0.0)
            wps = psum_pool.tile([P, NF], f32, name="ps", tag="ps")
            for _ in range(10):
                nc.tensor.matmul(wps[:], wz[:, :P], wz[:], start=True,
                                 stop=True)

            # L operands stream on TWO queues (gpsimd + vector), alternating
            # chunks: during the startup window the HBM arbiter splits
            # bandwidth per queue, and a single L queue measured only
            # ~100GB/s against the W slab queue's ~190GB/s, starving the
            # next step's L operand. Only L_0 is queued upfront (tapered);
            # L_i for i>=1 is issued inside the step loop one step ahead
            # of first use.
            def load_l(i, taper=False):
                k = 0
                n = 0
                while k < KT:
                    step = (1 if k < 2 else (2 if k < 8 else 4)) \
                        if taper else 4
                    eng = nc.gpsimd if n % 2 == 0 else nc.sync
                    eng.dma_start(lres[:, i, k:k + step],
                                  lt_t[:, i, k:k + step])
                    k += step
                    n += 1

            load_l(0, taper=True)

            def prefetch_slab(s, taper=False):
                ns, i = steps[s]
                nslc = slice(ns * NF, (ns + 1) * NF)
                slab = wts_pool.tile([P, KT, NF], bf16,
                                     name="wslab", tag="wslab")
                k = 0
                while k < KT:
                    step = (1 if k < 2 else (2 if k < 8 else 4)) \
                        if taper else 8
                    nc.scalar.dma_start(slab[:, k:k + step],
                                        rt_t[:, i, k:k + step, nslc])
                    k += step
                return slab

            slabs = {0: prefetch_slab(0, taper=True)}
            # bias is first needed ~14us in (first C init); queue it behind
            # the startup-critical first slab
            nc.scalar.dma_start(bias[:], bb[:])
            slabs[1] = prefetch_slab(1)

            cacc = {}    # (block, t) -> C accumulator tile
            for s, (ns, i) in enumerate(steps):
                slab = slabs.pop(s)
                # W prefetch depth 2 while L still streams (so the W queue
                # doesn't out-arbitrate L at the HBM), 3 afterwards
                want = (s + 2,) if s < 6 else \
                    ((s + 2, s + 3) if s == 6 else (s + 3,))
                for w in want:
                    if w < len(steps) and w not in slabs:
                        slabs[w] = prefetch_slab(w)
                if s < 6:
                    # stagger the remaining L loads one step ahead of use
                    load_l(s + 1)
                nh_cols = {0: slice(ns * NF, (ns + 1) * NF),
                           1: slice(N2 + ns * NF, N2 + (ns + 1) * NF)}
                last_step = s == len(steps) - 1
                # interleave the t-chains so back-to-back matmuls hit
                # different PSUM banks (same-bank accumulation turnaround
                # costs ~48ns/matmul otherwise). On the last step, two
                # pairs instead of one 4-wide group, so the first pair's
                # final combines+writebacks hide under the second pair.
                tgroups = [(0, 1), (2, 3)] if last_step else [(0, 1, 2, 3)]
                psums = {}
                for tg in tgroups:
                    for t in tg:
                        psums[t] = psum_pool.tile([P, NF], f32,
                                                  name="ps", tag="ps")
                    for kt in range(KT):
                        for t in tg:
                            nc.tensor.matmul(
                                psums[t][:],
                                lres[:, i, kt, t * P:(t + 1) * P],
                                slab[:, kt],
                                start=(kt == 0),
                                stop=(kt == KT - 1))
                    for t in tg:
                        ps = psums[t]
                        for blk, action in _CONTRIB[i]:
                            mh, nh = _BLOCK_POS[blk]
                            done = action.endswith("!")
                            op = SUB if action.startswith("sub") else ADD
                            if action == "init":
                                c = cacc_pool.tile([P, NF], f32,
                                                   name="c", tag="c")
                                cacc[(blk, t)] = c
                                nc.vector.tensor_tensor(
                                    c[:], ps[:], bias[:, nh_cols[nh]], ADD)
                            else:
                                c = cacc[(blk, t)]
                                if done and last_step and t == MT2 - 1:
                                    # the very last eviction is on the
                                    # critical path: split halves so the
                                    # first writeback overlaps the second
                                    # combine
                                    h = NF // 2
                                    for hs in (slice(0, h), slice(h, NF)):
                                        nc.vector.tensor_tensor(
                                            c[:, hs], c[:, hs], ps[:, hs],
                                            op)
                                        ncol = nh_cols[nh]
                                        sub = slice(ncol.start + hs.start,
                                                    ncol.start + hs.stop)
                                        nc.sync.dma_start(
                                            out_t[:, mh * MT2 + t, sub],
                                            c[:, hs])
                                    continue
                                nc.vector.tensor_tensor(c[:], c[:], ps[:],
                                                        op)
                            if done:
                                nc.sync.dma_start(
                                    out_t[:, mh * MT2 + t, nh_cols[nh]],
                                    c[:])

    nc.compile()
    return nc


def prepare_in_maps(x, W, b):
    bf16 = ml_dtypes.bfloat16
    x = np.asarray(x, dtype=np.float32)
    W = np.asarray(W, dtype=np.float32)
    b = np.asarray(b, dtype=np.float32)

    Wt = W.T  # [IN, OUT]
    W11, W12 = Wt[:K2, :N2], Wt[:K2, N2:]
    W21, W22 = Wt[K2:, :N2], Wt[K2:, N2:]
    R = np.stack([W11 + W22, W11, W12 - W22, W21 - W11, W22,
                  W11 + W12, W21 + W22]).astype(bf16)     # [7, K2, N2]
    R = np.ascontiguousarray(R)
    bias = np.ascontiguousarray(
        np.broadcast_to(b[None, :], (P, OUT))).astype(bf16)

    in_maps = []
    for c in range(NCORES):
        xs = x[c * MS:(c + 1) * MS]
        X11, X12 = xs[:M2, :K2], xs[:M2, K2:]
        X21, X22 = xs[M2:, :K2], xs[M2:, K2:]
        # transposed to contraction-major [K2, M2]
        L = np.stack([(X11 + X22).T, (X21 + X22).T, X11.T, X22.T,
                      (X11 + X12).T, (X21 - X11).T,
                      (X12 - X22).T]).astype(bf16)         # [7, K2, M2]
        in_maps.append({"lt": np.ascontiguousarray(L), "rt": R,
                        "bb": bias})
    return in_maps


def kernel(x, W, b):
    from concourse.bass_utils import run_bass_kernel_spmd

    nc = _cache.get("nc")
    if nc is None:
        nc = _cache["nc"] = _build()

    res = run_bass_kernel_spmd(nc, prepare_in_maps(x, W, b),
                               list(range(NCORES)))
    return np.concatenate(
        [res.results[c]["out"] for c in range(NCORES)], axis=0)


# revision 16
# speedup vs baseline: 1.0321x; 1.0321x over previous
"""Dense linear layer out = x @ W.T + b on 8 Trainium2 NeuronCores.

Strategy: data-parallel over the batch dim (8192/8 = 1024 rows per core),
W replicated, plus single-level Strassen per core to cut TensorE work to
7/8 of the naive roofline (the baseline bf16 kernel already ran at ~95%
of the 437us/core peak, so fewer MACs is the only lever left).

Per core, with 2x2x2 blocking of [1024,4096]@[4096,4096]:
    M1=(X11+X22)(W11+W22)  M2=(X21+X22)W11  M3=X11(W12-W22)
    M4=X22(W21-W11)        M5=(X11+X12)W22  M6=(X21-X11)(W11+W12)
    M7=(X12-X22)(W21+W22)
    C11=M1+M4-M5+M7  C12=M3+M5  C21=M2+M4  C22=M1-M2+M3+M6

All 7 lhs operands L_i (x-side sums, [2048,512] each) and 7 rhs operands
R_i (W-side sums, [2048,2048]) are precomputed on the HOST in fp32 and
shipped as bf16 - the device never does input additions. L is SBUF
resident (112KB/partition-col); R streams through a 3-slot slab ring.
Each M_i[t] tile is accumulated in PSUM (16-matmul chain) and folded
into the C-block SBUF accumulators by vector-engine ops directly from
PSUM (bias is folded into the first contribution), so M_i is never
stored. 1792 matmuls of [128x128]@[128x512] vs 2048 for the direct
kernel. Error ~4.6e-3 (bf16 operand sums double the bf16 noise).
"""

import numpy as np
import ml_dtypes

B, IN, OUT = 8192, 4096, 4096
NCORES = 8
MS = B // NCORES    # 1024 batch rows per core

P = 128
NF = 512            # matmul moving free dim (one PSUM bank of fp32)
M2 = MS // 2        # 512   Strassen half-M
K2 = IN // 2        # 2048  half-K
N2 = OUT // 2       # 2048  half-N
KT = K2 // P        # 16 contraction k-tiles per product
MT2 = M2 // P       # 4 m-tiles per half
NSL = N2 // NF      # 4 column slabs per product

_cache = {}

# per-product contributions: i -> list of (block, action)
# actions: 'init' = C := psum + bias ; 'add'/'sub' = C op= psum;
# a trailing '!' marks the block complete -> DMA out
_CONTRIB = {
    0: [("C11", "init"), ("C22", "init")],
    1: [("C21", "init"), ("C22", "sub")],
    2: [("C12", "init"), ("C22", "add")],
    3: [("C11", "add"), ("C21", "add!")],
    4: [("C11", "sub"), ("C12", "add!")],
    5: [("C22", "add!")],
    6: [("C11", "add!")],
}
# block -> (m-half, n-half)
_BLOCK_POS = {"C11": (0, 0), "C12": (0, 1), "C21": (1, 0), "C22": (1, 1)}


def _build():
    import concourse.mybir as mybir
    import concourse.tile as tile
    from concourse import bacc

    bf16 = mybir.dt.bfloat16
    f32 = mybir.dt.float32
    ADD = mybir.AluOpType.add
    SUB = mybir.AluOpType.subtract

    nc = bacc.Bacc("TRN2", target_bir_lowering=False, debug=False,
                   num_devices=NCORES)
    # SBUF-order host layouts: per partition row the whole slab is
    # contiguous (16KB), minimizing DMA descriptor work per transfer
    lt = nc.dram_tensor("lt", [7, P, KT, M2], bf16, kind="ExternalInput")
    rt = nc.dram_tensor("rt", [7, NSL, P, KT, NF], bf16,
                        kind="ExternalInput")
    bb = nc.dram_tensor("bb", [P, OUT], bf16, kind="ExternalInput")
    out = nc.dram_tensor("out", [MS, OUT], f32, kind="ExternalOutput")

    lt_t = lt[:].rearrange("i p kt m -> p i kt m")      # [128,7,KT,M2]
    rt_t = rt[:].rearrange("i ns p kt n -> p i ns kt n")   # [128,7,NSL,KT,NF]
    out_t = out[:].rearrange("(mt p) n -> p mt n", p=P)    # [128,8,OUT]

    steps = [(ns, i) for ns in range(NSL) for i in range(7)]

    with tile.TileContext(nc) as tc:
        with (
            tc.tile_pool(name="lres", bufs=1) as lres_pool,
            tc.tile_pool(name="bias", bufs=1) as bias_pool,
            tc.tile_pool(name="wts", bufs=3) as wts_pool,
            tc.tile_pool(name="psum", bufs=8, space="PSUM") as psum_pool,
            tc.tile_pool(name="cacc", bufs=16) as cacc_pool,
        ):
            lres = lres_pool.tile([P, 7, KT, M2], bf16)   # 112KB/partition
            bias = bias_pool.tile([P, OUT], bf16)

            # PE warmup: burn the NEFF-preamble + first-DMA window with
            # dummy matmuls so the clock gate opens before the real stream.
            wz = bias_pool.tile([P, NF], bf16, name="wz")
            nc.vector.memset(wz[:], 0.0)
            wps = psum_pool.tile([P, NF], f32, name="ps", tag="ps")
            for _ in range(24):
                nc.tensor.matmul(wps[:], wz[:, :P], wz[:], start=True,
                                 stop=True)

            # L operands stream on TWO queues (gpsimd + vector), alternating
            # chunks: during the startup window the HBM arbiter splits
            # bandwidth per queue, and a single L queue measured only
            # ~100GB/s against the W slab queue's ~190GB/s, starving the
            # next step's L operand. Only L_0 is queued upfront (tapered);
            # L_i for i>=1 is issued inside the step loop one step ahead
            # of first use.
            def load_l(i, taper=False):
                k = 0
                n = 0
                while k < KT:
                    step = (1 if k < 2 else (2 if k < 8 else 4)) \
                        if taper else 4
                    eng = nc.gpsimd if n % 2 == 0 else nc.sync
                    eng.dma_start(lres[:, i, k:k + step],
                                  lt_t[:, i, k:k + step])
                    k += step
                    n += 1

            load_l(0, taper=True)

            def prefetch_slab(s, taper=False):
                ns, i = steps[s]
                slab = wts_pool.tile([P, KT, NF], bf16,
                                     name="wslab", tag="wslab")
                k = 0
                while k < KT:
                    step = (1 if k < 2 else (2 if k < 8 else 4)) \
                        if taper else 8
                    nc.scalar.dma_start(slab[:, k:k + step],
                                        rt_t[:, i, ns, k:k + step])
                    k += step
                return slab

            slabs = {0: prefetch_slab(0, taper=True)}
            # bias is first needed ~14us in (first C init); queue it behind
            # the startup-critical first slab
            nc.scalar.dma_start(bias[:], bb[:])
            slabs[1] = prefetch_slab(1)

            cacc = {}    # (block, t) -> C accumulator tile
            for s, (ns, i) in enumerate(steps):
                slab = slabs.pop(s)
                # W prefetch depth 2 while L still streams (so the W queue
                # doesn't out-arbitrate L at the HBM), 3 afterwards
                want = (s + 2,) if s < 6 else \
                    ((s + 2, s + 3) if s == 6 else (s + 3,))
                for w in want:
                    if w < len(steps) and w not in slabs:
                        slabs[w] = prefetch_slab(w)
                if s < 6:
                    # stagger the remaining L loads one step ahead of use
                    load_l(s + 1)
                nh_cols = {0: slice(ns * NF, (ns + 1) * NF),
                           1: slice(N2 + ns * NF, N2 + (ns + 1) * NF)}
                last_step = s == len(steps) - 1
                # interleave the t-chains so back-to-back matmuls hit
                # different PSUM banks (same-bank accumulation turnaround
                # costs ~48ns/matmul otherwise). On the last step, two
                # pairs instead of one 4-wide group, so the first pair's
                # final combines+writebacks hide under the second pair.
                tgroups = [(0, 1), (2, 3)] if last_step else [(0, 1, 2, 3)]
                psums = {}
                for tg in tgroups:
                    for t in tg:
                        psums[t] = psum_pool.tile([P, NF], f32,
                                                  name="ps", tag="ps")
                    for kt in range(KT):
                        for t in tg:
                            nc.tensor.matmul(
                                psums[t][:],
                                lres[:, i, kt, t * P:(t + 1) * P],
                                slab[:, kt],
                                start=(kt == 0),
                                stop=(kt == KT - 1))
                    for t in tg:
                        ps = psums[t]
                        for blk, action in _CONTRIB[i]:
                            mh, nh = _BLOCK_POS[blk]
                            done = action.endswith("!")
                            op = SUB if action.startswith("sub") else ADD
                            if action == "init":
                                c = cacc_pool.tile([P, NF], f32,
                                                   name="c", tag="c")
                                cacc[(blk, t)] = c
                                nc.vector.tensor_tensor(
                                    c[:], ps[:], bias[:, nh_cols[nh]], ADD)
                            else:
                                c = cacc[(blk, t)]
                                if done and last_step and t == MT2 - 1:
                                    # the very last eviction is on the
                                    # critical path: split halves so the
                                    # first writeback overlaps the second
                                    # combine
                                    h = NF // 2
                                    for hs in (slice(0, h), slice(h, NF)):
                                        nc.vector.tensor_tensor(
                                            c[:, hs], c[:, hs], ps[:, hs],
                                            op)
                                        ncol = nh_cols[nh]
                                        sub = slice(ncol.start + hs.start,
                                                    ncol.start + hs.stop)
                                        nc.sync.dma_start(
                                            out_t[:, mh * MT2 + t, sub],
                                            c[:, hs])
                                    continue
                                nc.vector.tensor_tensor(c[:], c[:], ps[:],
                                                        op)
                            if done:
                                nc.sync.dma_start(
                                    out_t[:, mh * MT2 + t, nh_cols[nh]],
                                    c[:])

    nc.compile()
    return nc


def prepare_in_maps(x, W, b):
    bf16 = ml_dtypes.bfloat16
    x = np.asarray(x, dtype=np.float32)
    W = np.asarray(W, dtype=np.float32)
    b = np.asarray(b, dtype=np.float32)

    Wt = W.T  # [IN, OUT]
    W11, W12 = Wt[:K2, :N2], Wt[:K2, N2:]
    W21, W22 = Wt[K2:, :N2], Wt[K2:, N2:]
    R = np.stack([W11 + W22, W11, W12 - W22, W21 - W11, W22,
                  W11 + W12, W21 + W22]).astype(bf16)     # [7, K2, N2]
    # SBUF-order: [7, NSL, P, KT, NF] so each (i, ns) slab is contiguous
    # 16KB per partition row
    R = np.ascontiguousarray(
        R.reshape(7, KT, P, NSL, NF).transpose(0, 3, 2, 1, 4))
    bias = np.ascontiguousarray(
        np.broadcast_to(b[None, :], (P, OUT))).astype(bf16)

    in_maps = []
    for c in range(NCORES):
        xs = x[c * MS:(c + 1) * MS]
        X11, X12 = xs[:M2, :K2], xs[:M2, K2:]
        X21, X22 = xs[M2:, :K2], xs[M2:, K2:]
        # transposed to contraction-major [K2, M2]
        L = np.stack([(X11 + X22).T, (X21 + X22).T, X11.T, X22.T,
                      (X11 + X12).T, (X21 - X11).T,
                      (X12 - X22).T]).astype(bf16)         # [7, K2, M2]
        # SBUF-order: [7, P, KT, M2] so each L_i is contiguous per
        # partition row
        L = np.ascontiguousarray(
            L.reshape(7, KT, P, M2).transpose(0, 2, 1, 3))
        in_maps.append({"lt": L, "rt": R, "bb": bias})
    return in_maps


def kernel(x, W, b):
    from concourse.bass_utils import run_bass_kernel_spmd

    nc = _cache.get("nc")
    if nc is None:
        nc = _cache["nc"] = _build()

    res = run_bass_kernel_spmd(nc, prepare_in_maps(x, W, b),
                               list(range(NCORES)))
    return np.concatenate(
        [res.results[c]["out"] for c in range(NCORES)], axis=0)
